# revision 23
# baseline (speedup 1.0000x reference)
"""Group-quantized linear (fake int4 per-group dequant) GEMV on 8 Trainium2 cores.

Reference computation (all fp32):
    qw = round_half_even(clip(W, -8, 7))            # W in [-8, 7) so clip is identity
    out = (qw.reshape(O, 64, 128) * scales[:, :, None]).reshape(O, O) @ x

Sharding: column-parallel — each core owns a 1024-row slice of W/scales,
x replicated, outputs concatenated (per the tensor-parallel hint).

Key ideas:
- qw is a small-integer tensor (ints in [-8, 7]) which fp8e4m3 represents
  EXACTLY; quantization is computed bit-exactly on the host (same fp32
  round-half-even as the reference) and shipped as fp8, cutting HBM weight
  traffic 4x (32 MiB -> 8 MiB/core).  HW exec is then HBM-streaming-bound.
- The GEMV is restructured so the WEIGHTS are the PE's moving operand and a
  block-diagonal x is the stationary operand; with perf_mode=DoubleRow the
  fp8 array virtualizes to 128x256 and streams TWO weight channels per
  column-cycle, keeping the TensorEngine well under the DMA rate:

    per step u (32 steps), output chunk oc (2):
      lhsT [Ki=128, Ko=2, M=128] = xblk: row (g,cb), col (g',h) ->
           delta(g,g') * x8_h[g*128 + cb*64 + 2u + ko]     (h = Dekker hi/lo)
      rhs  [Ki=128, Ko=2, N=512] = qw[o, g*128 + cb*64 + 2u + ko]  (fp8)
      psum_oc[(g,h), o] += sum_{cb,ko} x8_h[...] * qw[o, ...]

  x is split x = hi + lo with both parts e4m3 (Dekker), recovering ~8
  mantissa bits; with exact int weights this lands at ~2e-3 rel err.
- All weight chunks get their own SBUF tiles (8 MiB fits SBUF) so every
  DMA is issued upfront and the ring streams continuously; a short burst
  of dummy matmuls warms the PE clock gate (HAM) during the pre-stream gap.
- Epilogue: z[(g,h), o] = psum * scalesT (scales duplicated over h on the
  host), then out[o] = ones[128].T @ z — partition reduction on the PE.
"""

import numpy as np
import ml_dtypes

IN_DIM = 8192
OUT_DIM = 8192
NG = 64  # quantization groups (128 channels each)
N_CORES = 8
PER_OUT = OUT_DIM // N_CORES  # 1024
P = 128
U = 32  # steps: each covers 4 channels/group = (cb in {0,1}) x (ko in {0,1})
OC_W = 512  # output chunk width (one PSUM bank)

_cache = {}

UCHUNKS = [2, 4, 4, 4, 4, 4, 4, 4, 1, 1]  # u-steps per weight DMA (sum 32)
U_SPLIT = 16  # accumulation split point: epilogue for u<16 runs mid-stream


def _split_multi_waits(nc):
    """walrus in this container accepts only ONE sync-wait per instruction;
    Tile's tail drain carries one per producer proc. Hoist extras onto
    same-engine NoOps placed immediately before — identical semantics for an
    in-order sequencer."""
    import concourse.mybir as mybir

    uid = 0
    for f in nc.m.functions:
        for blk in f.blocks:
            insts = blk.instructions
            if not any(
                i.sync_info is not None
                and i.sync_info.on_wait
                and len(i.sync_info.on_wait) > 1
                for i in insts
            ):
                continue
            new_insts = []
            for inst in insts:
                si = inst.sync_info
                if si is not None and si.on_wait and len(si.on_wait) > 1:
                    waits = list(si.on_wait)
                    for w in waits[:-1]:
                        uid += 1
                        new_insts.append(
                            mybir.InstNoOp(
                                name=f"I-waitsplit-{uid}",
                                engine=inst.engine,
                                ins=[],
                                outs=[],
                                sync_info=mybir.SyncInfo(on_wait=[w], on_update=[]),
                            )
                        )
                    inst.sync_info = mybir.SyncInfo(
                        on_wait=[waits[-1]], on_update=si.on_update
                    )
                new_insts.append(inst)
            blk.instructions = new_insts
    return nc


def build_nc(split_waits=True, n_warmup=8, uchunks=None):
    import concourse.bass as bass
    import concourse.mybir as mybir
    import concourse.tile as tile

    f32 = mybir.dt.float32
    bf16 = mybir.dt.bfloat16
    f8 = mybir.dt.float8e4
    mult = mybir.AluOpType.mult
    DR = mybir.MatmulPerfMode.DoubleRow

    if uchunks is None:
        uchunks = UCHUNKS
    assert sum(uchunks) == U

    nc = bass.Bass()
    # [(g,cb), u, ko, o] fp8: qw[o, g*128 + cb*64 + 2u + ko]
    wq = nc.dram_tensor("wq", [P, U, 2, PER_OUT], f8, kind="ExternalInput")
    # Dekker-split x values [128 (g,cb), 32 u, 2 ko, 2 h] fp8 (16 KiB) and
    # the h-duplicated block-diagonal mask [128, 128 (h,g')] fp8 (16 KiB);
    # the full 1 MiB block-diagonal lhsT is expanded on the idle DVE to
    # keep the HBM stream almost pure weights.
    vx_d = nc.dram_tensor("vx", [P, U, 2, 2], f8, kind="ExternalInput")
    xm_d = nc.dram_tensor("xm", [P, P], f8, kind="ExternalInput")
    # scales transposed + duplicated over h: [128 (h,g), 2 oc, 512 o'] bf16
    st_d = nc.dram_tensor("st", [P, 2, OC_W], bf16, kind="ExternalInput")
    out_d = nc.dram_tensor("out", [PER_OUT], f32, kind="ExternalOutput")

    with tile.TileContext(nc) as tc:
        with (
            tc.tile_pool(name="singles", bufs=1) as singles,
            tc.tile_pool(name="psum", bufs=1, space="PSUM") as psum,
        ):
            # ---- weight DMAs: every chunk gets its own SBUF tile (8 MiB
            # total fits SBUF) so ALL transfers are issued upfront and the
            # ring streams continuously with no buffer-reuse waits.
            ones = singles.tile([P, 1], bf16)
            nc.gpsimd.memset(ones, 1.0)
            # All weights stream on the sync HWDGE ring.
            wtiles = []
            u0 = 0
            for ci, clen in enumerate(uchunks):
                wt_ = singles.tile([P, clen, 2, PER_OUT], f8, name=f"w{ci}")
                nc.sync.dma_start(wt_, wq.ap()[:, u0 : u0 + clen, :, :])
                wtiles.append((u0, clen, wt_))
                u0 += clen

            # ---- aux loads ride the scalar HWDGE ring, in parallel
            vx = singles.tile([P, U, 2, 2], f8)
            nc.scalar.dma_start(vx, vx_d.ap())
            xm = singles.tile([P, P], f8)
            nc.scalar.dma_start(xm, xm_d.ap())
            st = singles.tile([P, 2, OC_W], bf16)
            nc.scalar.dma_start(st, st_d.ap())

            # ---- expand the block-diagonal lhsT on the DVE:
            # xblk[p, u, ko, (h,g')] = xm[p, (h,g')] * vx[p, u, ko, h]
            # (first small piece unblocks chunk-0 matmuls early)
            xblk = singles.tile([P, U, 2, P], f8)
            m3 = xm.rearrange("p (h g) -> p h g", h=2)
            for lo, hi in ((0, 2), (2, U)):
                n = hi - lo
                for ko in range(2):
                    nc.vector.tensor_tensor(
                        xblk[:, lo:hi, ko, :].rearrange(
                            "p u (h g) -> p u h g", h=2
                        ),
                        vx[:, lo:hi, ko, :].unsqueeze(3).broadcast_to(
                            [P, n, 2, NG]
                        ),
                        m3.unsqueeze(1).broadcast_to([P, n, 2, NG]),
                        mult,
                    )

            # ---- PE warm-up: dummy matmuls with no DMA dependency so the
            # HAM clock gate reaches K=8/8 before the real stream begins.
            wm_ps = psum.tile([1, OC_W], f32, tag="warm")
            for _ in range(n_warmup):
                nc.tensor.matmul(
                    wm_ps,
                    lhsT=ones[:, 0:1],
                    rhs=ones.broadcast_to([P, OC_W]),
                    start=True,
                    stop=True,
                )

            # ---- main: DoubleRow fp8 — 2 interleaved weights per PE cell.
            # Accumulation is split at U_SPLIT into separate psum banks so
            # the first half's scale+reduce epilogue runs mid-stream; the
            # reduce matmuls accumulate both halves into one psum [1, 512].
            accs = {
                (half, oc): psum.tile(
                    [P, OC_W], f32, tag=f"acc{half}{oc}", name=f"acc{half}{oc}"
                )
                for half in range(2)
                for oc in range(2)
            }
            oreds = [
                psum.tile([1, OC_W], f32, tag="ored0", name="ored0"),
                psum.tile([1, OC_W], f32, tag="ored1", name="ored1"),
            ]

            def epilogue(half, oc):
                z = singles.tile([P, OC_W], bf16, name=f"z{half}{oc}")
                nc.vector.tensor_tensor(z, accs[(half, oc)], st[:, oc, :], mult)
                nc.tensor.matmul(
                    oreds[oc],
                    lhsT=ones,
                    rhs=z,
                    start=(half == 0),
                    stop=(half == 1),
                )

            for u0, clen, wt_ in wtiles:
                for ul in range(clen):
                    u = u0 + ul
                    half = int(u >= U_SPLIT)
                    for oc in range(2):
                        nc.tensor.matmul(
                            accs[(half, oc)],
                            lhsT=xblk[:, u, :, :],
                            rhs=wt_[:, ul, :, oc * OC_W : (oc + 1) * OC_W],
                            start=(u % U_SPLIT == 0),
                            stop=(u % U_SPLIT == U_SPLIT - 1),
                            perf_mode=DR,
                        )
                    if u == U_SPLIT - 1:
                        epilogue(0, 0)
                        epilogue(0, 1)

            epilogue(1, 0)
            epilogue(1, 1)
            out_sb = singles.tile([1, PER_OUT], f32)
            nc.vector.tensor_copy(out=out_sb[:, 0:OC_W], in_=oreds[0])
            nc.scalar.copy(out=out_sb[:, OC_W:PER_OUT], in_=oreds[1])
            nc.sync.dma_start(out_d.rearrange("(a o) -> a o", a=1), out_sb)

    return _split_multi_waits(nc) if split_waits else nc


def _prep_inputs(x, weights, scales):
    """Host-side shard + layout. Quantization here is bit-exact vs the
    reference (same fp32 round-half-even; ints in [-8,7] are exact in fp8)."""
    f8t = ml_dtypes.float8_e4m3
    x = np.ascontiguousarray(np.asarray(x, dtype=np.float32))
    weights = np.asarray(weights, dtype=np.float32)
    scales = np.asarray(scales, dtype=np.float32)

    # Dekker split of x into two e4m3 parts: x ~ hi + lo
    xhi = x.astype(f8t).astype(np.float32)
    xlo = (x - xhi).astype(f8t).astype(np.float32)
    # channel index k = g*128 + cb*64 + 2u + ko  ->  [p=(g,cb), u, ko]
    xs = np.stack([xhi, xlo])  # [h, 8192]
    xs = xs.reshape(2, NG, 2, U, 2)  # [h, g, cb, u, ko]
    # vx[(g,cb), u, ko, h]; device expands vx*mask -> block-diagonal lhsT
    vx = np.ascontiguousarray(
        xs.transpose(1, 2, 3, 4, 0).reshape(P, U, 2, 2).astype(f8t)
    )
    # xm[(g,cb), h*64+g'] = delta(g,g')  (duplicated over h)
    eye2 = np.repeat(np.eye(NG, dtype=np.float32), 2, axis=0)  # [(g,cb), g']
    xm = np.ascontiguousarray(
        np.concatenate([eye2, eye2], axis=1).astype(f8t)
    )  # [128, 128]

    in_maps = []
    for c in range(N_CORES):
        sl = slice(c * PER_OUT, (c + 1) * PER_OUT)
        qw = np.rint(np.clip(weights[sl], -8.0, 7.0))
        # [o, (g, cb, u, ko)] -> [(g, cb), u, ko, o]
        wqa = (
            qw.reshape(PER_OUT, NG, 2, U, 2)
            .transpose(1, 2, 3, 4, 0)
            .reshape(P, U, 2, PER_OUT)
        )
        wqa = np.ascontiguousarray(wqa).astype(f8t)
        s_t = scales[sl].T  # [g, o] = [64, 1024]
        # st[h*64+g, oc, o'] = scales[oc*512+o', g]  (same for h=0,1)
        s_go = np.stack([s_t[:, :OC_W], s_t[:, OC_W:]], axis=1)  # [g, oc, o']
        st = np.ascontiguousarray(
            np.concatenate([s_go, s_go], axis=0).astype(ml_dtypes.bfloat16)
        )  # [128, 2, 512] bf16
        in_maps.append({"wq": wqa, "vx": vx, "xm": xm, "st": st})
    return in_maps


def kernel(x, weights, scales):
    from concourse import bass_utils

    if "nc" not in _cache:
        _cache["nc"] = build_nc()
    nc = _cache["nc"]

    in_maps = _prep_inputs(x, weights, scales)
    res = bass_utils.run_bass_kernel_spmd(nc, in_maps, core_ids=list(range(N_CORES)))
    return np.concatenate([res.results[c]["out"] for c in range(N_CORES)]).astype(
        np.float32
    )


# revision 24
# speedup vs baseline: 1.0645x; 1.0645x over previous
"""Group-quantized linear (fake int4 per-group dequant) GEMV on 8 Trainium2 cores.

Reference computation (all fp32):
    qw = round_half_even(clip(W, -8, 7))            # W in [-8, 7) so clip is identity
    out = (qw.reshape(O, 64, 128) * scales[:, :, None]).reshape(O, O) @ x

Sharding: column-parallel — each core owns a 1024-row slice of W/scales,
x replicated, outputs concatenated (per the tensor-parallel hint).

Key ideas:
- qw is a small-integer tensor (ints in [-8, 7]) which fp8e4m3 represents
  EXACTLY; quantization is computed bit-exactly on the host (same fp32
  round-half-even as the reference) and shipped as fp8, cutting HBM weight
  traffic 4x (32 MiB -> 8 MiB/core).  HW exec is then HBM-streaming-bound.
- The GEMV is restructured so the WEIGHTS are the PE's moving operand and a
  block-diagonal x is the stationary operand; with perf_mode=DoubleRow the
  fp8 array virtualizes to 128x256 and streams TWO weight channels per
  column-cycle, keeping the TensorEngine well under the DMA rate:

    per step u (32 steps), output chunk oc (2):
      lhsT [Ki=128, Ko=2, M=128] = xblk: row (g,cb), col (g',h) ->
           delta(g,g') * x8_h[g*128 + cb*64 + 2u + ko]     (h = Dekker hi/lo)
      rhs  [Ki=128, Ko=2, N=512] = qw[o, g*128 + cb*64 + 2u + ko]  (fp8)
      psum_oc[(g,h), o] += sum_{cb,ko} x8_h[...] * qw[o, ...]

  x is split x = hi + lo with both parts e4m3 (Dekker), recovering ~8
  mantissa bits; with exact int weights this lands at ~2e-3 rel err.
- All weight chunks get their own SBUF tiles (8 MiB fits SBUF) so every
  DMA is issued upfront and the ring streams continuously; a short burst
  of dummy matmuls warms the PE clock gate (HAM) during the pre-stream gap.
- Epilogue: z[(g,h), o] = psum * scalesT (scales duplicated over h on the
  host), then out[o] = ones[128].T @ z — partition reduction on the PE.
"""

import numpy as np
import ml_dtypes

IN_DIM = 8192
OUT_DIM = 8192
NG = 64  # quantization groups (128 channels each)
N_CORES = 8
PER_OUT = OUT_DIM // N_CORES  # 1024
P = 128
U = 32  # steps: each covers 4 channels/group = (cb in {0,1}) x (ko in {0,1})
OC_W = 512  # output chunk width (one PSUM bank)

_cache = {}

UCHUNKS = [2, 4, 4, 4, 4, 4, 4, 4, 1, 1]  # u-steps per weight DMA (sum 32)
U_SPLIT = 16  # accumulation split point: epilogue for u<16 runs mid-stream


def _split_multi_waits(nc):
    """walrus in this container accepts only ONE sync-wait per instruction;
    Tile's tail drain carries one per producer proc. Hoist extras onto
    same-engine NoOps placed immediately before — identical semantics for an
    in-order sequencer."""
    import concourse.mybir as mybir

    uid = 0
    for f in nc.m.functions:
        for blk in f.blocks:
            insts = blk.instructions
            if not any(
                i.sync_info is not None
                and i.sync_info.on_wait
                and len(i.sync_info.on_wait) > 1
                for i in insts
            ):
                continue
            new_insts = []
            for inst in insts:
                si = inst.sync_info
                if si is not None and si.on_wait and len(si.on_wait) > 1:
                    waits = list(si.on_wait)
                    for w in waits[:-1]:
                        uid += 1
                        new_insts.append(
                            mybir.InstNoOp(
                                name=f"I-waitsplit-{uid}",
                                engine=inst.engine,
                                ins=[],
                                outs=[],
                                sync_info=mybir.SyncInfo(on_wait=[w], on_update=[]),
                            )
                        )
                    inst.sync_info = mybir.SyncInfo(
                        on_wait=[waits[-1]], on_update=si.on_update
                    )
                new_insts.append(inst)
            blk.instructions = new_insts
    return nc


def build_nc(split_waits=True, n_warmup=8, uchunks=None):
    import concourse.bass as bass
    import concourse.mybir as mybir
    import concourse.tile as tile

    f32 = mybir.dt.float32
    bf16 = mybir.dt.bfloat16
    f8 = mybir.dt.float8e4
    mult = mybir.AluOpType.mult
    DR = mybir.MatmulPerfMode.DoubleRow

    if uchunks is None:
        uchunks = UCHUNKS
    assert sum(uchunks) == U

    nc = bass.Bass()
    # [(g,cb), u, ko, o] fp8: qw[o, g*128 + cb*64 + 2u + ko]
    wq = nc.dram_tensor("wq", [P, U, 2, PER_OUT], f8, kind="ExternalInput")
    # Dekker-split x values [128 (g,cb), 32 u, 2 ko, 2 h] fp8 (16 KiB) and
    # the h-duplicated block-diagonal mask [128, 128 (h,g')] fp8 (16 KiB);
    # the full 1 MiB block-diagonal lhsT is expanded on the idle DVE to
    # keep the HBM stream almost pure weights.
    vx_d = nc.dram_tensor("vx", [P, U, 2, 2], f8, kind="ExternalInput")
    xm_d = nc.dram_tensor("xm", [P, P], f8, kind="ExternalInput")
    # scales transposed + duplicated over h: [128 (h,g), 2 oc, 512 o'] bf16
    st_d = nc.dram_tensor("st", [P, 2, OC_W], bf16, kind="ExternalInput")
    out_d = nc.dram_tensor("out", [PER_OUT], f32, kind="ExternalOutput")

    with tile.TileContext(nc) as tc:
        with (
            tc.tile_pool(name="singles", bufs=1) as singles,
            tc.tile_pool(name="psum", bufs=1, space="PSUM") as psum,
        ):
            # ---- weight DMAs: every chunk gets its own SBUF tile (8 MiB
            # total fits SBUF) so ALL transfers are issued upfront and the
            # ring streams continuously with no buffer-reuse waits.
            ones = singles.tile([P, 1], bf16)
            nc.gpsimd.memset(ones, 1.0)
            # All weights stream on the sync HWDGE ring.
            wtiles = []
            u0 = 0
            for ci, clen in enumerate(uchunks):
                wt_ = singles.tile([P, clen, 2, PER_OUT], f8, name=f"w{ci}")
                nc.sync.dma_start(wt_, wq.ap()[:, u0 : u0 + clen, :, :])
                wtiles.append((u0, clen, wt_))
                u0 += clen

            # ---- aux loads ride the scalar HWDGE ring, in parallel
            vx = singles.tile([P, U, 2, 2], f8)
            nc.scalar.dma_start(vx, vx_d.ap())
            xm = singles.tile([P, P], f8)
            nc.scalar.dma_start(xm, xm_d.ap())
            st = singles.tile([P, 2, OC_W], bf16)
            nc.scalar.dma_start(st, st_d.ap())

            # ---- expand the block-diagonal lhsT:
            # xblk[p, u, ko, (h,g')] = xm[p, (h,g')] * vx[p, u, ko, h]
            # split by u-range across DVE and GpSimd so the pieces build in
            # parallel (~2 us each) and chunk-0's piece lands first.
            xblk = singles.tile([P, U, 2, P], f8)
            m3 = xm.rearrange("p (h g) -> p h g", h=2)
            for lo, hi, eng in (
                (0, 2, nc.vector),
                (2, 10, nc.gpsimd),
                (10, 18, nc.vector),
                (18, 25, nc.gpsimd),
                (25, U, nc.vector),
            ):
                n = hi - lo
                for ko in range(2):
                    eng.tensor_tensor(
                        xblk[:, lo:hi, ko, :].rearrange(
                            "p u (h g) -> p u h g", h=2
                        ),
                        vx[:, lo:hi, ko, :].unsqueeze(3).broadcast_to(
                            [P, n, 2, NG]
                        ),
                        m3.unsqueeze(1).broadcast_to([P, n, 2, NG]),
                        mult,
                    )

            # ---- PE warm-up: dummy matmuls with no DMA dependency so the
            # HAM clock gate reaches K=8/8 before the real stream begins.
            wm_ps = psum.tile([1, OC_W], f32, tag="warm")
            for _ in range(n_warmup):
                nc.tensor.matmul(
                    wm_ps,
                    lhsT=ones[:, 0:1],
                    rhs=ones.broadcast_to([P, OC_W]),
                    start=True,
                    stop=True,
                )

            # ---- main: DoubleRow fp8 — 2 interleaved weights per PE cell.
            # Accumulation is split at U_SPLIT into separate psum banks so
            # the first half's scale+reduce epilogue runs mid-stream; the
            # reduce matmuls accumulate both halves into one psum [1, 512].
            accs = {
                (half, oc): psum.tile(
                    [P, OC_W], f32, tag=f"acc{half}{oc}", name=f"acc{half}{oc}"
                )
                for half in range(2)
                for oc in range(2)
            }
            oreds = [
                psum.tile([1, OC_W], f32, tag="ored0", name="ored0"),
                psum.tile([1, OC_W], f32, tag="ored1", name="ored1"),
            ]

            def epilogue(half, oc):
                z = singles.tile([P, OC_W], bf16, name=f"z{half}{oc}")
                nc.vector.tensor_tensor(z, accs[(half, oc)], st[:, oc, :], mult)
                nc.tensor.matmul(
                    oreds[oc],
                    lhsT=ones,
                    rhs=z,
                    start=(half == 0),
                    stop=(half == 1),
                )

            for u0, clen, wt_ in wtiles:
                for ul in range(clen):
                    u = u0 + ul
                    half = int(u >= U_SPLIT)
                    for oc in range(2):
                        nc.tensor.matmul(
                            accs[(half, oc)],
                            lhsT=xblk[:, u, :, :],
                            rhs=wt_[:, ul, :, oc * OC_W : (oc + 1) * OC_W],
                            start=(u % U_SPLIT == 0),
                            stop=(u % U_SPLIT == U_SPLIT - 1),
                            perf_mode=DR,
                        )
                    if u == U_SPLIT + 7:
                        # half-A epilogue, emitted a couple of chunks after
                        # half A completed so its DVE z is long done and the
                        # reduce matmul slots into the PE queue bubble-free
                        epilogue(0, 0)
                        epilogue(0, 1)

            epilogue(1, 0)
            epilogue(1, 1)
            out_sb = singles.tile([1, PER_OUT], f32)
            nc.vector.tensor_copy(out=out_sb[:, 0:OC_W], in_=oreds[0])
            nc.scalar.copy(out=out_sb[:, OC_W:PER_OUT], in_=oreds[1])
            nc.sync.dma_start(out_d.rearrange("(a o) -> a o", a=1), out_sb)

    return _split_multi_waits(nc) if split_waits else nc


def _prep_inputs(x, weights, scales):
    """Host-side shard + layout. Quantization here is bit-exact vs the
    reference (same fp32 round-half-even; ints in [-8,7] are exact in fp8)."""
    f8t = ml_dtypes.float8_e4m3
    x = np.ascontiguousarray(np.asarray(x, dtype=np.float32))
    weights = np.asarray(weights, dtype=np.float32)
    scales = np.asarray(scales, dtype=np.float32)

    # Dekker split of x into two e4m3 parts: x ~ hi + lo
    xhi = x.astype(f8t).astype(np.float32)
    xlo = (x - xhi).astype(f8t).astype(np.float32)
    # channel index k = g*128 + cb*64 + 2u + ko  ->  [p=(g,cb), u, ko]
    xs = np.stack([xhi, xlo])  # [h, 8192]
    xs = xs.reshape(2, NG, 2, U, 2)  # [h, g, cb, u, ko]
    # vx[(g,cb), u, ko, h]; device expands vx*mask -> block-diagonal lhsT
    vx = np.ascontiguousarray(
        xs.transpose(1, 2, 3, 4, 0).reshape(P, U, 2, 2).astype(f8t)
    )
    # xm[(g,cb), h*64+g'] = delta(g,g')  (duplicated over h)
    eye2 = np.repeat(np.eye(NG, dtype=np.float32), 2, axis=0)  # [(g,cb), g']
    xm = np.ascontiguousarray(
        np.concatenate([eye2, eye2], axis=1).astype(f8t)
    )  # [128, 128]

    in_maps = []
    for c in range(N_CORES):
        sl = slice(c * PER_OUT, (c + 1) * PER_OUT)
        qw = np.rint(np.clip(weights[sl], -8.0, 7.0))
        # [o, (g, cb, u, ko)] -> [(g, cb), u, ko, o]
        wqa = (
            qw.reshape(PER_OUT, NG, 2, U, 2)
            .transpose(1, 2, 3, 4, 0)
            .reshape(P, U, 2, PER_OUT)
        )
        wqa = np.ascontiguousarray(wqa).astype(f8t)
        s_t = scales[sl].T  # [g, o] = [64, 1024]
        # st[h*64+g, oc, o'] = scales[oc*512+o', g]  (same for h=0,1)
        s_go = np.stack([s_t[:, :OC_W], s_t[:, OC_W:]], axis=1)  # [g, oc, o']
        st = np.ascontiguousarray(
            np.concatenate([s_go, s_go], axis=0).astype(ml_dtypes.bfloat16)
        )  # [128, 2, 512] bf16
        in_maps.append({"wq": wqa, "vx": vx, "xm": xm, "st": st})
    return in_maps


def kernel(x, weights, scales):
    from concourse import bass_utils

    if "nc" not in _cache:
        _cache["nc"] = build_nc()
    nc = _cache["nc"]

    in_maps = _prep_inputs(x, weights, scales)
    res = bass_utils.run_bass_kernel_spmd(nc, in_maps, core_ids=list(range(N_CORES)))
    return np.concatenate([res.results[c]["out"] for c in range(N_CORES)]).astype(
        np.float32
    )
